# revision 1
# baseline (speedup 1.0000x reference)
"""Trainium2 Bass kernel for BinarizeConv2dSDP.

Math (reference):
    s   = M + rv @ Z          (the rsqrt normalization is sign-preserving:
                               w = (m + rv@z) * rsqrt(...) with rsqrt > 0,
                               so sign(w) == sign(s))
    bw  = sign(s)             (O, I, 3, 3)
    ba  = sign(x)             (B, C, H, W)
    out = conv2d(ba, bw, stride 1, pad 1) * Alpha

Strategy:
    - Data-parallel over batch: 8 cores x 4 images each. M/Z/Alpha replicated.
    - Weight synthesis on-device per core: 5 fused (z*rv_k)+prev ops; each
      full-width op (~1.4us) pipelines behind the per-Z DMA wire (~1.7us),
      then sign -> 9 PE transposes -> 2 packing copies.
    - Binarized conv: sign(x) stored fp8e4 in a zero-padded [128, 58 x 64]
      SBUF image (row stride 64 so a vertical tap pair is a 64B step).
      Per output row-block, 3 DoubleRow matmuls (vertical tap pairs, K=256)
      + 3 normal fp8 matmuls (ky=2 taps) accumulate into PSUM. +-1 is exact
      in fp8e4/bf16 and PSUM accumulates in f32, so results are exact.
    - All input DMAs are issued up front (x0 between the weight loads);
      output stores follow on the same queue, sem-gated per evacuation.
    - Alpha applied during PSUM->SBUF evacuation; f32 out. Bit-equal to the
      reference modulo conv summation order (integer-exact).
"""

import os
import numpy as np

import concourse.bass as bass
import concourse.tile as tile
from concourse import bacc, mybir
from concourse.bass_utils import run_bass_kernel_spmd
from concourse.masks import make_identity

F32 = mybir.dt.float32
BF16 = mybir.dt.bfloat16
FP8 = mybir.dt.float8e4

USE_FP8 = bool(int(os.environ.get("BASS_KERNEL_FP8", "1")))

B_FULL = 32
N_CORES = 8
B_CORE = B_FULL // N_CORES  # 4 images per core
C = 128      # in channels
O = 128      # out channels
H = W = 56
HP = 58                      # padded rows
WP = 64 if USE_FP8 else 58   # padded row stride (64 -> tap-pair step is 64B)
KS = 3
NTAPS = KS * KS
IKK = C * NTAPS  # 1152
ROWS_PER_TILE = 8           # output rows per PSUM tile -> N = 8*56 = 448
N_TILE = ROWS_PER_TILE * W  # 448 fp32 <= 512 (one PSUM bank)
N_ROW_TILES = H // ROWS_PER_TILE  # 7
ADT = FP8 if USE_FP8 else BF16


def build_program(rv: np.ndarray, n_img: int = B_CORE):
    """Build the per-core Bass program. rv values are baked as immediates."""
    nc = bacc.Bacc(
        "TRN2",
        target_bir_lowering=False,
        debug=False,
        num_devices=N_CORES,
    )

    x_t = nc.dram_tensor("x", (n_img, C, H, W), F32, kind="ExternalInput").ap()
    a_t = nc.dram_tensor("Alpha", (O, 1, 1), F32, kind="ExternalInput").ap()
    m_t = nc.dram_tensor("M", (O, C, KS, KS), F32, kind="ExternalInput").ap()
    z_t = nc.dram_tensor("Z", (5, O, C, KS, KS), F32, kind="ExternalInput").ap()
    out_t = nc.dram_tensor("out", (n_img, O, H, W), F32, kind="ExternalOutput").ap()

    rv = np.asarray(rv, dtype=np.float32).reshape(-1)
    assert rv.shape[0] == 5

    with tile.TileContext(nc) as tc:
        with (
            tc.tile_pool(name="const", bufs=1) as const_pool,
            tc.tile_pool(name="wsyn", bufs=1) as wsyn_pool,
            tc.tile_pool(name="imgs", bufs=1) as img_pool,
            tc.tile_pool(name="xstage", bufs=4) as x_pool,
            tc.tile_pool(name="evac", bufs=8) as ev_pool,
            tc.tile_pool(name="cpsum", bufs=6, space="PSUM") as cpsum_pool,
            tc.tile_pool(name="tpsum", bufs=1, space="PSUM") as tpsum_pool,
        ):
            # --- x0 first on the wire: its sign hides under the Z DMAs ---
            alpha_sb = const_pool.tile([O, 1], F32)
            nc.sync.dma_start(alpha_sb, a_t.rearrange("o a b -> o (a b)"))
            x_tiles = [None] * n_img
            x_tiles[0] = x_pool.tile([C, H * W], F32, name="x0", tag="xin")
            nc.sync.dma_start(x_tiles[0], x_t[0].rearrange("c h w -> c (h w)"))

            identity = const_pool.tile([128, 128], BF16)
            make_identity(nc, identity)

            m_sb = wsyn_pool.tile([O, IKK], F32)
            nc.sync.dma_start(m_sb, m_t.rearrange("o i kh kw -> o (i kh kw)"))
            z_sbs = []
            for k in range(5):
                z_sb = wsyn_pool.tile([O, IKK], F32, name=f"z{k}", tag=f"z{k}")
                nc.sync.dma_start(
                    z_sb, z_t[k].rearrange("o i kh kw -> o (i kh kw)")
                )
                z_sbs.append(z_sb)

            # --- per-image padded sign(x) buffers (borders zeroed once) ---
            padded = []
            for img in range(n_img):
                pd = img_pool.tile(
                    [C, HP * WP], ADT, name=f"pad{img}", tag=f"pad{img}"
                )
                pd3 = pd.rearrange("p (h w) -> p h w", w=WP)
                nc.gpsimd.memset(pd3[:, 0, 0:HP], 0.0)
                nc.gpsimd.memset(pd3[:, HP - 1, 0:HP], 0.0)
                nc.gpsimd.memset(pd3[:, 1 : HP - 1, 0:1], 0.0)
                nc.gpsimd.memset(pd3[:, 1 : HP - 1, HP - 1 : HP], 0.0)
                padded.append(pd3)

            def sign_image(img):
                pd3 = padded[img]
                nc.scalar.sign(
                    pd3[:, 1 : 1 + H, 1 : 1 + W],
                    x_tiles[img].rearrange("c (h w) -> c h w", w=W),
                )

            sign_image(0)

            # --- weight synthesis: s = M + sum_k rv_k Z_k.
            # The tail after Z4 lands is chunked over the free (i) dim so
            # sign/transposes of earlier chunks overlap the last stt ops;
            # every op still spans all 128 partitions (full engine lanes).
            NCHUNK, CCH = 4, 32
            s_sb = wsyn_pool.tile([O, IKK], F32)
            bw_nat = wsyn_pool.tile([O, IKK], BF16)
            bw3 = bw_nat.rearrange("o (i t) -> o i t", t=NTAPS)
            if USE_FP8:
                bw_pair = wsyn_pool.tile([C, KS, 2, O], FP8)
                bw_single = wsyn_pool.tile([C, KS, O], FP8)
                tpP = tpsum_pool.tile([128, KS * 2 * O], BF16)
                tpS = tpsum_pool.tile([128, KS * O], BF16)
            else:
                bw_lhsT = wsyn_pool.tile([C, NTAPS, O], BF16)
                tpP = tpsum_pool.tile([128, 4 * O], BF16)
                tpS = tpsum_pool.tile([128, 5 * O], BF16)
            for ic in range(NCHUNK):
                csl = slice(ic * CCH * NTAPS, (ic + 1) * CCH * NTAPS)
                for k in range(5):
                    nc.vector.scalar_tensor_tensor(
                        out=s_sb[:, csl],
                        in0=z_sbs[k][:, csl],
                        scalar=float(rv[k]),
                        in1=(m_sb if k == 0 else s_sb)[:, csl],
                        op0=mybir.AluOpType.mult,
                        op1=mybir.AluOpType.add,
                    )
                nc.scalar.sign(bw_nat[:, csl], s_sb[:, csl])
                psl = slice(ic * CCH, (ic + 1) * CCH)
                for t in range(NTAPS):
                    ky, kx = divmod(t, KS)
                    if USE_FP8:
                        dst, toff = (
                            (tpP, (kx * 2 + ky) * O) if ky < 2 else (tpS, kx * O)
                        )
                    else:
                        dst, toff = (tpP, t * O) if t < 4 else (tpS, (t - 4) * O)
                    nc.tensor.transpose(
                        dst[psl, toff : toff + O],
                        bw3[:, psl, t],
                        identity,
                        tile_position=(0, ic * CCH),
                    )
            if USE_FP8:
                nc.scalar.copy(
                    bw_pair.rearrange("p a b o -> p (a b o)"), tpP
                )
                nc.vector.tensor_copy(
                    bw_single.rearrange("p a o -> p (a o)"), tpS
                )
            else:
                nc.vector.tensor_copy(
                    bw_lhsT[:, 0:4, :],
                    tpP.rearrange("p (t o) -> p t o", o=O),
                )
                nc.vector.tensor_copy(
                    bw_lhsT[:, 4:NTAPS, :],
                    tpS.rearrange("p (t o) -> p t o", o=O),
                )

            # --- main conv loop; next image's load+sign emitted before this
            # image's tiles so ACT never head-of-line blocks the sign ---
            for img in range(n_img):
                if img + 1 < n_img:
                    nxt = img + 1
                    x_tiles[nxt] = x_pool.tile(
                        [C, H * W], F32, name=f"x{nxt}", tag="xin"
                    )
                    nc.sync.dma_start(
                        x_tiles[nxt], x_t[nxt].rearrange("c h w -> c (h w)")
                    )
                    sign_image(nxt)
                pd3 = padded[img]

                for nt in range(N_ROW_TILES):
                    y0 = nt * ROWS_PER_TILE
                    cv = cpsum_pool.tile([O, N_TILE], F32, tag="cv")
                    if USE_FP8:
                        for kx in range(KS):
                            win0 = pd3[:, y0 : y0 + ROWS_PER_TILE, kx : kx + W]
                            ap4 = bass.AP(
                                win0.tensor,
                                win0.offset,
                                [list(win0.ap[0]), [WP, 2]]
                                + [list(p) for p in win0.ap[1:]],
                            )
                            nc.tensor.matmul(
                                cv,
                                bw_pair[:, kx],
                                ap4,
                                start=(kx == 0),
                                stop=False,
                                perf_mode=mybir.MatmulPerfMode.DoubleRow,
                            )
                        for kx in range(KS):
                            win = pd3[
                                :, y0 + 2 : y0 + 2 + ROWS_PER_TILE, kx : kx + W
                            ]
                            nc.tensor.matmul(
                                cv,
                                bw_single[:, kx],
                                win,
                                start=False,
                                stop=(kx == KS - 1),
                            )
                    else:
                        t = 0
                        for ky in range(KS):
                            for kx in range(KS):
                                win = pd3[
                                    :,
                                    y0 + ky : y0 + ky + ROWS_PER_TILE,
                                    kx : kx + W,
                                ]
                                nc.tensor.matmul(
                                    cv,
                                    bw_lhsT[:, t, :],
                                    win,
                                    start=(t == 0),
                                    stop=(t == NTAPS - 1),
                                )
                                t += 1
                    ev = ev_pool.tile([O, N_TILE], F32, tag="ev")
                    nc.vector.tensor_scalar_mul(ev, cv, alpha_sb[:, 0:1])
                    # stores on their own queues: never head-of-line block
                    # the x loads riding the sync queue
                    dma_eng = nc.scalar if (nt % 2 == 0) else nc.gpsimd
                    dma_eng.dma_start(
                        out_t[img, :, y0 : y0 + ROWS_PER_TILE, :],
                        ev.rearrange("o (h w) -> o h w", w=W),
                    )

    nc.compile()
    return nc


def _ensure_ntff_hook():
    """Register the axon NTFF profiling hook if the image's antenv lacks it.

    Only used when BASS_KERNEL_TRACE=1 (dev profiling); best-effort.
    """
    import sys
    import types

    try:
        import antenv

        if hasattr(antenv, "axon_hooks"):
            return
        mod = types.ModuleType("antenv.axon_hooks")
        _hook = [None]
        mod.set_axon_ntff_profile_hook = lambda h: _hook.__setitem__(0, h)
        mod.get_axon_ntff_profile_hook = lambda: _hook[0]
        sys.modules["antenv.axon_hooks"] = mod
        antenv.axon_hooks = mod
        from trn_agent_boot.trn_boot import _ntff_profile_via_ctypes

        mod.set_axon_ntff_profile_hook(
            _ntff_profile_via_ctypes("/opt/axon/libaxon_pjrt.so")
        )
    except Exception as e:  # pragma: no cover - profiling is optional
        print(f"NTFF hook registration failed ({e}); tracing disabled")


def kernel(x, Alpha, M, Z, rv):
    x = np.ascontiguousarray(np.asarray(x, dtype=np.float32))
    Alpha = np.ascontiguousarray(np.asarray(Alpha, dtype=np.float32))
    M = np.ascontiguousarray(np.asarray(M, dtype=np.float32))
    Z = np.ascontiguousarray(np.asarray(Z, dtype=np.float32))
    rv = np.asarray(rv, dtype=np.float32)

    trace = bool(int(os.environ.get("BASS_KERNEL_TRACE", "0")))
    if trace:
        _ensure_ntff_hook()

    nc = build_program(rv)

    in_maps = []
    for c in range(N_CORES):
        in_maps.append(
            {
                "x": np.ascontiguousarray(x[c * B_CORE : (c + 1) * B_CORE]),
                "Alpha": Alpha,
                "M": M,
                "Z": Z,
            }
        )

    res = run_bass_kernel_spmd(
        nc,
        in_maps,
        core_ids=list(range(N_CORES)),
        trace=trace,
    )
    out = np.concatenate([res.results[c]["out"] for c in range(N_CORES)], axis=0)
    if trace:
        kernel.last_results = res
    return out

